# revision 16
# baseline (speedup 1.0000x reference)
"""Trainium2 Bass kernel for nn_Decoder2 (GRU decoder, Keras reset_after GRUCell).

Reference computation (per batch row b, scanned over t = 0..T-1):
    x_t   = [o_{t-1}, feat_t]                  # [1+F]
    mx    = x_t @ K + ib                       # [3H]
    mh    = h_{t-1} @ Wr + rb                  # [3H]
    z     = sigmoid(mx[:H]   + mh[:H])
    r     = sigmoid(mx[H:2H] + mh[H:2H])
    cand  = tanh(mx[2H:] + r * mh[2H:])
    h_t   = z * h_{t-1} + (1-z) * cand
    o_t   = h_t @ dw + db                      # scalar output per row

Shapes: B=8192, T=96, F=64, H=256.

Strategy: pure data parallel over batch (1024 rows/core on 8 cores), no
collectives.  On-chip layout is feature-major: [feature -> partitions,
batch -> free dim]; two batch tiles of n=512 pipeline against each other.

Key structural points (vs a naive per-step loop):
  * x-prepass: the x-side contribution (feat rows + o row) of step t+1
    is accumulated into each gate's PSUM bank (start=True) as soon as
    the bank is freed by step t's consumer, so the only matmuls on the
    recurrence critical path are the h passes (start=False, stop=True).
    This also keeps the PE queue occupied, holding the p-state clock up.
  * No weight folding: the o-feedback rides as a bf16 x-row exactly
    like the reference (a fold would bury the rank-1 dw@k0 term in
    bf16/fp8 rounding of Wr).  The dense output is computed into the
    freed hh PSUM slot, ACT-copied to SBUF fp32 (PSUM cannot source a
    DMA), DMA'd to DRAM, and Pool-copied into the next x-tile's o-row.
  * Engine balance: ACT does sig(r), sig(z), tanh, o-copy; DVE does
    rh = hh*r, the carg add, and the three h_new ops (4x mode, all-SBUF
    bf16); Pool only does the tiny o-row copy.
  * fp8 DoubleRow was evaluated and rejected: DR matmuls may only write
    PSUM partition 0..M-1 (ISA s3d3_mm_valid_dst_partition), so 256-row
    gate outputs cannot use it without halving PSUM capacity or
    elementwise lane utilization.
"""

import os
import sys

for _p in ("/root/.axon_site/_ro/trn_rl_repo", "/opt/trn_rl_repo"):
    if os.path.isdir(_p) and _p not in sys.path:
        sys.path.insert(0, _p)

from contextlib import ExitStack  # noqa: E402

import numpy as np  # noqa: E402

import concourse.bacc as bacc  # noqa: E402
import concourse.tile as tile  # noqa: E402
from concourse import mybir  # noqa: E402
from concourse import bass_utils  # noqa: E402

Alu = mybir.AluOpType
Act = mybir.ActivationFunctionType

B, T, F, H = 8192, 96, 64, 256
G3 = 3 * H              # 768 gate width
NCORES = 8
BL = B // NCORES        # 1024 batch rows per core
OROW = F                # x-tile row carrying o_{t-1}


def build_nc(
    t_steps: int = T,
    bl: int = BL,
    nt: int = 2,
    compute_dt: str = "bfloat16",
    with_bias: bool = False,
    prepass: bool = True,
    debug_dump: bool = False,
):
    """Build (and compile) the per-core Bass program.

    with_bias: include a ones-row in the x-tiles carrying
    (ib + rb_zr + db*k0); the graded inputs have all-zero biases so the
    default fast path drops it.
    """
    del compute_dt
    n = bl // nt
    assert n <= 512
    nch = H // 128
    assert nch == 2
    f32 = mybir.dt.float32
    bf16 = mybir.dt.bfloat16
    XR = F + 1 + (1 if with_bias else 0)   # feat rows + o row (+ ones row)

    nc = bacc.Bacc("TRN2", target_bir_lowering=False, debug=False)

    featT = nc.dram_tensor("featT", [t_steps, F, bl], bf16, kind="ExternalInput").ap()
    h0T = nc.dram_tensor("h0T", [128, nch, bl], bf16, kind="ExternalInput").ap()
    o0 = nc.dram_tensor("o0", [1, bl], bf16, kind="ExternalInput").ap()
    # x-side stationary: rows 0..63 = kx[1:], row 64 = k0 (o row)
    kxw = nc.dram_tensor("kxw", [XR, G3], bf16, kind="ExternalInput").ap()
    wrw = nc.dram_tensor("wrw", [128, nch, G3], bf16, kind="ExternalInput").ap()
    dww = nc.dram_tensor("dww", [128, nch], bf16, kind="ExternalInput").ap()
    rbh = nc.dram_tensor("rbh", [128, nch], f32, kind="ExternalInput").ap()
    outT = nc.dram_tensor("outT", [t_steps, bl], f32, kind="ExternalOutput").ap()
    if debug_dump:
        dbg = {k: nc.dram_tensor(f"dbg_{k}", [128, nch, bl], f32,
                                 kind="ExternalOutput").ap()
               for k in ("zp", "rp", "hh", "xh", "carg", "hn")}

    with tile.TileContext(nc) as tc, ExitStack() as ctx:
        const = ctx.enter_context(tc.tile_pool(name="const", bufs=1))
        hpool = ctx.enter_context(tc.tile_pool(name="h", bufs=3))
        xpool = ctx.enter_context(tc.tile_pool(name="x", bufs=6))
        zpool = ctx.enter_context(tc.tile_pool(name="z", bufs=3))
        rpool = ctx.enter_context(tc.tile_pool(name="r", bufs=3))
        apool = ctx.enter_context(tc.tile_pool(name="a", bufs=3))
        bpool = ctx.enter_context(tc.tile_pool(name="b", bufs=3))
        rhpool = ctx.enter_context(tc.tile_pool(name="rh", bufs=3))
        cpool = ctx.enter_context(tc.tile_pool(name="cand", bufs=3))
        opool = ctx.enter_context(tc.tile_pool(name="osb", bufs=4))
        pool_dump = ctx.enter_context(tc.tile_pool(name="dmp", bufs=2)) if debug_dump else None
        pz = ctx.enter_context(tc.tile_pool(name="pz", bufs=1, space="PSUM"))
        pr = ctx.enter_context(tc.tile_pool(name="pr", bufs=1, space="PSUM"))
        phh = ctx.enter_context(tc.tile_pool(name="phh", bufs=1, space="PSUM"))
        pxh = ctx.enter_context(tc.tile_pool(name="pxh", bufs=1, space="PSUM"))

        # --- constants ---
        kx_sb = const.tile([XR, G3], bf16)
        nc.sync.dma_start(out=kx_sb, in_=kxw)
        wr_sb = const.tile([128, nch, G3], bf16)
        nc.sync.dma_start(out=wr_sb, in_=wrw)
        dw_sb = const.tile([128, nch], bf16)
        nc.sync.dma_start(out=dw_sb, in_=dww)
        rb_sb = const.tile([128, nch], f32)
        nc.sync.dma_start(out=rb_sb, in_=rbh)

        # --- initial state ---
        h_prev = hpool.tile([128, nch, bl], bf16)
        nc.sync.dma_start(out=h_prev, in_=h0T)

        xs = {}
        for tt in range(min(2, t_steps)):
            for j in range(nt):
                xj = xpool.tile([XR, n], bf16, tag="x", name="xj")
                nc.sync.dma_start(out=xj[0:F, :],
                                  in_=featT[tt, :, j * n:(j + 1) * n])
                if with_bias:
                    nc.gpsimd.memset(xj[F + 1:F + 2, :], 1.0)
                if tt == 0:
                    nc.sync.dma_start(out=xj[OROW:OROW + 1, :],
                                      in_=o0[0:1, j * n:(j + 1) * n])
                xs[(tt, j)] = xj

        def dump(key, src_ap, bs):
            if not debug_dump:
                return
            dt_ = pool_dump.tile([128, nch, n], f32, tag="dmp", name="dt_")
            nc.scalar.activation(dt_, src_ap, Act.Copy)
            nc.sync.dma_start(out=dbg[key][:, :, bs], in_=dt_)
        # gate column chunk of gate gi is at gi*H + ci*128
        def x_prepass(ps, gi, t, j):
            """Accumulate the x contribution of gate gi for step t into
            psum tile ps (start=True)."""
            x = xs[(t, j)]
            for ci in range(nch):
                m = gi * H + ci * 128
                nc.tensor.matmul(ps[:, ci, :], kx_sb[:, m:m + 128], x,
                                 start=True, stop=False)

        def h_pass(ps, gi, h, j, first=False):
            """Accumulate the h contribution of gate gi (stop=True).
            first=True when no x-prepass seeded the bank (the hh gate),
            so the c==0 matmul must start the accumulation group."""
            for ci in range(nch):
                m = gi * H + ci * 128
                for c in range(nch):
                    nc.tensor.matmul(
                        ps[:, ci, :], wr_sb[:, c, m:m + 128],
                        h[:, c, j * n:(j + 1) * n],
                        start=first and c == 0, stop=c == nch - 1)

        h_hist = {-1: h_prev}
        pzs, prs, pxhs = {}, {}, {}

        # prime step-0 psum tiles + x prepasses
        for j in range(nt if prepass else 0):
            pzs[(0, j)] = pz.tile([128, nch, n], f32, tag="pz", name="pz0")
            prs[(0, j)] = pr.tile([128, nch, n], f32, tag="pr", name="pr0")
            pxhs[(0, j)] = pxh.tile([128, nch, n], f32, tag="pxh", name="pxh0")
            x_prepass(prs[(0, j)], 1, 0, j)
            x_prepass(pzs[(0, j)], 0, 0, j)
            x_prepass(pxhs[(0, j)], 2, 0, j)

        for t in range(t_steps):
            h_new = hpool.tile([128, nch, bl], bf16, tag="h")
            h_hist[t] = h_new
            h_prev = h_hist[t - 1]

            state = {}
            # --- phase A: gate matmuls + sigmoids ---
            for j in range(nt):
                if prepass:
                    rp = prs.pop((t, j))
                else:
                    rp = pr.tile([128, nch, n], f32, tag="pr", name="rpi")
                    x_prepass(rp, 1, t, j)
                h_pass(rp, 1, h_prev, j)
                if t == 0:
                    dump("rp", rp, slice(j * n, (j + 1) * n))
                r_sb = rpool.tile([128, nch, n], bf16, tag="r")
                nc.scalar.activation(r_sb, rp, Act.Sigmoid)

                if prepass:
                    zp = pzs.pop((t, j))
                else:
                    zp = pz.tile([128, nch, n], f32, tag="pz", name="zpi")
                    x_prepass(zp, 0, t, j)
                h_pass(zp, 0, h_prev, j)
                if t == 0:
                    dump("zp", zp, slice(j * n, (j + 1) * n))
                z_sb = zpool.tile([128, nch, n], bf16, tag="z")
                nc.scalar.activation(z_sb, zp, Act.Sigmoid)

                hhp = phh.tile([128, nch, n], f32, tag="phh")
                h_pass(hhp, 2, h_prev, j, first=True)
                if t == 0:
                    dump("hh", hhp, slice(j * n, (j + 1) * n))
                state[j] = (hhp, r_sb, z_sb)

            # --- phase B: rh, carg, tanh;  zr prepasses for t+1 ---
            cands = {}
            for j in range(nt):
                hhp, r_sb, z_sb = state[j]
                if prepass:
                    xhp = pxhs.pop((t, j))
                else:
                    xhp = pxh.tile([128, nch, n], f32, tag="pxh", name="xpi")
                    x_prepass(xhp, 2, t, j)
                rh_sb = rhpool.tile([128, nch, n], bf16, tag="rh")
                for ci in range(nch):
                    # rh = (hh + rb_h) * r   (rb_h is zeros w/o bias)
                    nc.vector.scalar_tensor_tensor(
                        rh_sb[:, ci, :], hhp[:, ci, :], rb_sb[:, ci:ci + 1],
                        r_sb[:, ci, :], Alu.add, Alu.mult)
                if t == 0:
                    dump("xh", xhp, slice(j * n, (j + 1) * n))
                nc.vector.tensor_tensor(xhp, xhp, rh_sb, Alu.add)
                if t == 0:
                    dump("carg", xhp, slice(j * n, (j + 1) * n))
                cand = cpool.tile([128, nch, n], bf16, tag="cand")
                nc.scalar.activation(cand, xhp, Act.Tanh)
                cands[j] = cand

            # --- phase C: h_new, dense, o feedback, xh prepass t+1 ---
            for j in range(nt):
                bs = slice(j * n, (j + 1) * n)
                hhp, r_sb, z_sb = state[j]
                cand = cands[j]
                a_sb = apool.tile([128, nch, n], bf16, tag="a")
                nc.vector.tensor_tensor(a_sb, z_sb, h_prev[:, :, bs], Alu.mult)
                b_sb = bpool.tile([128, nch, n], bf16, tag="b")
                nc.vector.scalar_tensor_tensor(
                    b_sb, z_sb, -1.0, cand, Alu.add, Alu.mult)
                nc.vector.tensor_tensor(h_new[:, :, bs], a_sb, b_sb,
                                        Alu.subtract)
                if t == 0:
                    dump("hn", h_new[:, :, bs], bs)

                # dense output into the freed hh bank slot
                op = phh.tile([1, n], f32, tag="phh")
                nc.tensor.matmul(op[0:1, :], dw_sb[:, 0:1],
                                 h_new[:, 0, bs], start=True, stop=False)
                nc.tensor.matmul(op[0:1, :], dw_sb[:, 1:2],
                                 h_new[:, 1, bs], start=False, stop=True)
                # PSUM cannot source a DMA: ACT stages fp32 in SBUF.
                o_sb = opool.tile([1, n], f32, tag="osb")
                nc.scalar.activation(o_sb, op, Act.Copy)
                nc.sync.dma_start(out=outT[t:t + 1, bs], in_=o_sb)

                if t < t_steps - 1:
                    # o feedback into next step's x-tile.  All t+1
                    # prepasses MUST be emitted after this write: program
                    # order defines the dataflow, so an earlier-emitted
                    # prepass would read a stale o-row.
                    nc.gpsimd.tensor_copy(out=xs[(t + 1, j)][OROW:OROW + 1, :],
                                          in_=o_sb)
                    if prepass:
                        rpn = pr.tile([128, nch, n], f32, tag="pr", name="prn")
                        zpn = pz.tile([128, nch, n], f32, tag="pz", name="pzn")
                        xpn = pxh.tile([128, nch, n], f32, tag="pxh", name="pxhn")
                        prs[(t + 1, j)] = rpn
                        pzs[(t + 1, j)] = zpn
                        pxhs[(t + 1, j)] = xpn
                        x_prepass(rpn, 1, t + 1, j)
                        x_prepass(zpn, 0, t + 1, j)
                        x_prepass(xpn, 2, t + 1, j)

                # prefetch feat for t+2
                if t < t_steps - 2:
                    xj = xpool.tile([XR, n], bf16, tag="x", name="xj")
                    nc.sync.dma_start(out=xj[0:F, :],
                                      in_=featT[t + 2, :, bs])
                    if with_bias:
                        nc.gpsimd.memset(xj[F + 1:F + 2, :], 1.0)
                    xs[(t + 2, j)] = xj

            h_hist.pop(t - 2, None)
            for j in range(nt):
                xs.pop((t - 1, j), None)

    nc.compile()
    return nc


_NC_CACHE: dict = {}


def _get_nc(t_steps=T, bl=BL, nt=2, compute_dt="bfloat16", with_bias=False,
            prepass=True):
    key = (t_steps, bl, nt, with_bias, prepass)
    if key not in _NC_CACHE:
        _NC_CACHE[key] = build_nc(t_steps, bl, nt, compute_dt, with_bias, prepass)
    return _NC_CACHE[key]


def make_in_maps(
    decoder_feature,
    init_state,
    decoder_init_input,
    kernel,
    recurrent_kernel,
    input_bias,
    recurrent_bias,
    dense_w,
    dense_b,
    t_steps=T,
    bl=BL,
    ncores=NCORES,
    with_bias=False,
):
    np_bf16 = mybir.dt.np(mybir.dt.bfloat16)
    f = np.asarray(decoder_feature, np.float32)
    h0 = np.asarray(init_state, np.float32)
    o0 = np.asarray(decoder_init_input, np.float32)
    kx = np.asarray(kernel, np.float32)
    wr = np.asarray(recurrent_kernel, np.float32)
    ib = np.asarray(input_bias, np.float32)
    rb = np.asarray(recurrent_bias, np.float32)
    dw = np.asarray(dense_w, np.float32)
    db = float(np.asarray(dense_b, np.float32).reshape(-1)[0])
    k0 = kx[0]

    XR = F + 1 + (1 if with_bias else 0)
    kxw = np.zeros((XR, G3), np.float32)
    kxw[0:F] = kx[1:]
    kxw[OROW] = k0
    if with_bias:
        # ones-row: ib everywhere + rb on the z/r columns (rb_h instead
        # rides inside the r* product via the stt).  The fed-back o-row
        # holds o_raw = h@dw (no db), so db routes through here as db*k0.
        ones = np.concatenate([(ib + rb)[:2 * H], ib[2 * H:]])
        kxw[F + 1] = ones + db * k0

    in_maps = []
    for i in range(ncores):
        s = slice(i * bl, (i + 1) * bl)
        in_maps.append({
            "featT": np.ascontiguousarray(
                f[s, :t_steps].transpose(1, 2, 0)).astype(np_bf16),
            "h0T": np.ascontiguousarray(
                h0[s].T.reshape(2, 128, bl).transpose(1, 0, 2)).astype(np_bf16),
            "o0": np.ascontiguousarray(o0[s].T).astype(np_bf16),
            "kxw": kxw.astype(np_bf16),
            "wrw": np.ascontiguousarray(
                wr.reshape(2, 128, G3).transpose(1, 0, 2)).astype(np_bf16),
            "dww": np.ascontiguousarray(dw.reshape(2, 128).T).astype(np_bf16),
            "rbh": np.ascontiguousarray(
                rb[2 * H:].reshape(2, 128).T).astype(np.float32),
        })
    return in_maps, db


def run(inputs: dict, compute_dt="bfloat16", nt=2, trace=False, trace_kwargs=None):
    t_steps = int(inputs.get("predict_seq_length", T))
    assert t_steps == T, f"kernel hardcodes T={T}, got {t_steps}"
    ib = np.asarray(inputs["input_bias"], np.float32)
    rb = np.asarray(inputs["recurrent_bias"], np.float32)
    db = float(np.asarray(inputs["dense_b"], np.float32).reshape(-1)[0])
    with_bias = bool(np.any(ib) or np.any(rb) or db != 0.0)
    nc = _get_nc(T, BL, nt, compute_dt, with_bias)
    in_maps, db = make_in_maps(
        inputs["decoder_feature"], inputs["init_state"],
        inputs["decoder_init_input"], inputs["kernel"],
        inputs["recurrent_kernel"], inputs["input_bias"],
        inputs["recurrent_bias"], inputs["dense_w"], inputs["dense_b"],
        with_bias=with_bias,
    )
    res = bass_utils.run_bass_kernel_spmd(
        nc, in_maps, core_ids=list(range(NCORES)), trace=trace,
        **(trace_kwargs or {}),
    )
    out = np.empty((B, T, 1), np.float32)
    for i in range(NCORES):
        out[i * BL:(i + 1) * BL, :, 0] = res.results[i]["outT"].T + db
    return out, res


def kernel(**inputs) -> np.ndarray:
    out, _ = run(inputs)
    return out
